# revision 42
# baseline (speedup 1.0000x reference)
"""Multi-head attention (B=2,S=2048,D=1024,H=16) on 8 trn2 NeuronCores.

Sharding: core = b*4 + g  (b = batch 0..1, g = head-group 0..3, 4 heads each).
Each core computes QKV projections for its 256 output dims, causal attention
for its 4 heads (scores kept transposed: [s_k, s_q]), and a K-sliced partial
of the output projection (transposed: [D, S]).  Host sums the 4 partials per
batch and adds b_o.

All matmuls in bf16 (fp32 PSUM accumulate); softmax without max-subtraction
(scores/8 are small, exp cannot overflow); sumexp via an all-ones [128,64]
stationary matmul broadcast to the head's 64 partitions.

Schedule: one flat attention-tile stream with a 2-tile software-pipeline lag
between the score matmuls and the prob*V matmuls, so the PE never head-of-line
blocks on the ACT engine's exp.  Projection / output-projection matmul groups
are woven between tile slots from a static per-chunk plan.  PSUM: 2 score
buffers [128,1024] + 1 fused ctx|sumexp accumulator [128,1024] + 2 projection
buffers [128,512] = 8 banks.  Inputs are host-prepacked so each (tensor,
quarter) is a single 1MB contiguous-read DMA.
"""
import sys

if "/opt/trn_rl_repo" not in sys.path:
    sys.path.insert(0, "/opt/trn_rl_repo")

import numpy as np
import ml_dtypes

B, S, D, H = 2, 2048, 1024, 16
HD = D // H            # 64
G = 4                  # head groups (one per core within a batch)
HPG = H // G           # 4 heads per group
DG = HPG * HD          # 256 dims per group
SCALE = 8.0
NCORES = 8
NQC = S // 512         # 4 query chunks
NJ = S // 128          # 16 key tiles
KC = D // 128          # 8 contraction chunks
BF16 = ml_dtypes.bfloat16

_CACHE = {}


def _build(causal: bool):
    import concourse.mybir as mybir
    import concourse.tile as tile
    from concourse import bacc
    from collections import deque

    f32 = mybir.dt.float32
    b16 = mybir.dt.bfloat16
    Exp = mybir.ActivationFunctionType.Exp

    nc = bacc.Bacc(None, target_bir_lowering=False)

    # inputs host-prepacked: q/k/v as [128, quarter, kc*512], matching the
    # SBUF tile layout exactly so each (tensor, quarter) DMA is one fully
    # contiguous 8KB-per-partition transfer
    qT = nc.dram_tensor("qT", [128, NQC, KC * 512], b16, kind="ExternalInput")
    kT = nc.dram_tensor("kT", [128, NQC, KC * 512], b16, kind="ExternalInput")
    vT = nc.dram_tensor("vT", [128, NQC, KC * 512], b16, kind="ExternalInput")
    # weights host-prepacked to the exact SBUF tile layout (one DMA each)
    # wq/wk m-major ([128, m, kc*128]) so the first-needed m=0 half is its
    # own contiguous DMA on the critical path to the first score tile
    wqT = nc.dram_tensor("wqT", [128, 2, KC * 128], b16, kind="ExternalInput")
    wkT = nc.dram_tensor("wkT", [128, 2, KC * 128], b16, kind="ExternalInput")
    wvT = nc.dram_tensor("wvT", [128, KC * DG], b16, kind="ExternalInput")
    woT = nc.dram_tensor("woT", [128, 2 * D], b16, kind="ExternalInput")
    bq = nc.dram_tensor("bq", [128, 2], f32, kind="ExternalInput")
    bk = nc.dram_tensor("bk", [128, 2], f32, kind="ExternalInput")
    bv = nc.dram_tensor("bv", [1, DG], b16, kind="ExternalInput")
    tri = nc.dram_tensor("tri", [128, 128], b16, kind="ExternalInput")
    out = nc.dram_tensor("out", [D, S], b16, kind="ExternalOutput")

    with tile.TileContext(nc) as tc:
        with (
            tc.tile_pool(name="consts", bufs=1) as consts,
            tc.tile_pool(name="proj", bufs=1) as proj,
            tc.tile_pool(name="pin", bufs=1) as pin,
            tc.tile_pool(name="probs", bufs=8) as probsp,
            tc.tile_pool(name="rec", bufs=2) as recp,
            tc.tile_pool(name="ost", bufs=1) as ostp,
            tc.tile_pool(name="scp", bufs=2, space="PSUM") as scp,
            tc.tile_pool(name="csp", bufs=1, space="PSUM") as csp,
            tc.tile_pool(name="pjp", bufs=2, space="PSUM") as pjp,
        ):
            # --- constant tiles -------------------------------------------
            wq_t = consts.tile([128, KC * DG], b16)
            wk_t = consts.tile([128, KC * DG], b16)
            wv_t = consts.tile([128, KC * DG], b16)
            wo_t = consts.tile([128, 2 * D], b16)
            bq_t = consts.tile([128, 2], f32)
            bk_t = consts.tile([128, 2], f32)
            bv_t = consts.tile([1, DG], b16)
            tri_t = consts.tile([128, 128], b16)
            ones64_t = consts.tile([128, HD], b16)
            ones1_t = consts.tile([1, 128], b16)
            warm_sb = consts.tile([128, 128], b16)

            # --- persistent projection outputs ----------------------------
            # qpT/kpT: pair p in cols [p*S,(p+1)*S); rows 0:64 head 2p, 64:128 head 2p+1
            qpT = proj.tile([128, 2 * S], b16)
            kpT = proj.tile([128, 2 * S], b16)
            # vp: key tile j in cols [j*DG,(j+1)*DG); within: local head hh at 64*hh
            vp = proj.tile([128, NJ * DG], b16)
            # ctxT: same pair layout as qpT, normalized attention output (c x s)
            ctxT = proj.tile([128, 2 * S], b16)

            # --- input tiles: col = qtr*4096 + kc*512 + seq%512 -----------
            qb = pin.tile([128, KC * S], b16)
            kb = pin.tile([128, KC * S], b16)
            vb = pin.tile([128, KC * S], b16)

            def qcol(kc, n):
                # column of the 512-wide (contraction-chunk kc, quarter n) block
                return n * (KC * 512) + kc * 512

            def dma_qtr(eng, dst, src, qtr):
                eng.dma_start(dst[:, qtr * (KC * 512):(qtr + 1) * (KC * 512)], src[:, qtr])

            # all large inputs on the sync ring in strict first-use order:
            # within one queue's ring DMAs drain serially, so ring order IS
            # the bandwidth priority.  Tiny tensors ride gpsimd in parallel.
            # (Inputs must stay off the scalar queue - it paces softmax.)
            nc.sync.dma_start(wq_t[:], wqT.rearrange("p m c -> p (m c)"))
            dma_qtr(nc.sync, qb, qT, 0)
            nc.gpsimd.dma_start(bq_t[:], bq[:])
            nc.gpsimd.dma_start(bk_t[:], bk[:])
            nc.gpsimd.dma_start(bv_t[:], bv[:])
            nc.gpsimd.dma_start(tri_t[:], tri[:])
            nc.sync.dma_start(wk_t[:], wkT.rearrange("p m c -> p (m c)"))
            dma_qtr(nc.sync, kb, kT, 0)
            nc.sync.dma_start(wv_t[:], wvT[:])
            dma_qtr(nc.sync, vb, vT, 0)
            dma_qtr(nc.sync, qb, qT, 1)
            dma_qtr(nc.sync, kb, kT, 1)
            dma_qtr(nc.sync, vb, vT, 1)
            nc.sync.dma_start(wo_t[:], woT[:])
            for qtr in range(2, NQC):
                dma_qtr(nc.sync, qb, qT, qtr)
                dma_qtr(nc.sync, kb, kT, qtr)
                dma_qtr(nc.sync, vb, vT, qtr)

            nc.vector.memset(ones64_t[:], 1.0)
            nc.vector.memset(ones1_t[:], 1.0)
            nc.vector.memset(warm_sb[:], 0.0)

            # warmup burst: keeps the PE activity monitor at full clock
            # while the first input quarters stream in
            # sized to bridge until the first input quarter lands (~14.5us)
            # even when running entirely at the cold 1.2GHz clock, so the
            # HAM never re-throttles before the first projections
            warm_ps = scp.tile([128, 1024], f32, tag="sc", name="warm")
            for wi in range(78):
                nc.tensor.matmul(warm_ps[:, 0:128], warm_sb[:], warm_sb[:],
                                 start=(wi == 0), stop=(wi == 77))
            # preload the exp spline tables (~2.7us) during the DMA window
            nc.scalar.activation(warm_sb[:, 0:1], warm_sb[:, 0:1], Exp)

            # --- emitters --------------------------------------------------
            qk_open = {}

            def qk_proj_half(name, src, w_t, dst, bias_t, n, m, half):
                key = (name, n, m)
                if half == 0:
                    qk_open[key] = pjp.tile([128, 512], f32, tag="pj", name=f"{name}ps{n}{m}")
                ps = qk_open[key]
                for kc in range(4 * half, 4 * half + 4):
                    nc.tensor.matmul(
                        ps[:],
                        w_t[:, m * 1024 + kc * 128: m * 1024 + (kc + 1) * 128],
                        src[:, qcol(kc, n): qcol(kc, n) + 512],
                        start=(kc == 0), stop=(kc == KC - 1),
                    )
                if half == 1:
                    nc.vector.tensor_scalar_add(
                        dst[:, m * S + n * 512: m * S + (n + 1) * 512],
                        ps[:], bias_t[:, m:m + 1],
                    )
                    del qk_open[key]

            def v_proj_j(j):
                ps = pjp.tile([128, 512], f32, tag="pj", name=f"vps{j}")
                for kc in range(KC):
                    vcol = qcol(kc, j // 4) + (j % 4) * 128
                    nc.tensor.matmul(
                        ps[:, 0:DG], vb[:, vcol: vcol + 128],
                        wv_t[:, kc * DG:(kc + 1) * DG],
                        start=(kc == 0), stop=False,
                    )
                nc.tensor.matmul(ps[:, 0:DG], ones1_t[:], bv_t[:], start=False, stop=True)
                nc.vector.tensor_copy(vp[:, j * DG:(j + 1) * DG], ps[:, 0:DG])

            ostage = {}

            def oproj_dc(c, dc, ops=None):
                if ops is None:
                    ops = pjp.tile([128, 512], f32, tag="pj", name=f"op{c}{dc}")
                for p2 in range(2):
                    nc.tensor.matmul(
                        ops[:],
                        wo_t[:, p2 * D + dc * 128: p2 * D + (dc + 1) * 128],
                        ctxT[:, p2 * S + c * 512: p2 * S + (c + 1) * 512],
                        start=(p2 == 0), stop=(p2 == 1),
                    )
                # stage two 512-col chunks per output row-block so the
                # out-DMA moves 2KB contiguous bursts (bf16 [128,1024])
                if c % 2 == 0:
                    ostage[dc] = ostp.tile([128, 1024], b16, tag=f"ot{dc}", name=f"ot{c}{dc}")
                dst = ostage[dc][:, (c % 2) * 512:(c % 2 + 1) * 512]
                if c == NQC - 1 and dc % 2 == 0:
                    # softmax is done by now: split the tail staging copies
                    # across ACT and DVE so they drain twice as fast
                    nc.scalar.copy(dst, ops[:])
                else:
                    nc.vector.tensor_copy(dst, ops[:])
                if c % 2 == 1:
                    nc.sync.dma_start(
                        out[dc * 128:(dc + 1) * 128, (c - 1) * 512:(c + 1) * 512],
                        ostage[dc][:],
                    )

            def attn_j_sc(c, p, j):
                qoff = p * S + c * 512
                d = j - 4 * c if causal else -1
                coff = 0 if d < 0 else 128 * d
                sc = scp.tile([128, 1024], f32, tag="sc", name=f"sc{c}{p}{j}")
                for hh, (rlo, rhi) in enumerate(((0, 64), (64, 128))):
                    nc.tensor.matmul(
                        sc[:, hh * 512 + coff: hh * 512 + 512],
                        kpT[rlo:rhi, p * S + j * 128: p * S + (j + 1) * 128],
                        qpT[rlo:rhi, qoff + coff: qoff + 512],
                        start=True, stop=True, tile_position=(rlo, 0),
                    )
                pr = probsp.tile([128, 1024], b16, tag="pr", name=f"pr{c}{p}{j}")
                if coff == 0:
                    nc.scalar.activation(pr[:, 0:1024], sc[:, 0:1024], Exp, scale=1.0 / SCALE)
                else:
                    sc_v = sc.rearrange("p (h n) -> p h n", h=2)[:, :, coff:512]
                    pr_v = pr.rearrange("p (h n) -> p h n", h=2)[:, :, coff:512]
                    nc.scalar.activation(pr_v, sc_v, Exp, scale=1.0 / SCALE)
                if d >= 0:
                    for hh in range(2):
                        band = pr[:, hh * 512 + coff: hh * 512 + coff + 128]
                        nc.vector.tensor_mul(band, band, tri_t[:])
                return pr

            def attn_j_pv(c, p, j, nj, cs_t, pr):
                d = j - 4 * c if causal else -1
                coff = 0 if d < 0 else 128 * d
                first, last = (j == 0), (j == nj - 1)
                for hh in range(2):
                    prh = pr[:, hh * 512 + coff: hh * 512 + 512]
                    nc.tensor.matmul(
                        cs_t[hh * 64:(hh + 1) * 64, coff:512],
                        vp[:, j * DG + p * 128 + hh * 64: j * DG + p * 128 + (hh + 1) * 64],
                        prh, start=first, stop=last,
                        tile_position=(0, hh * 64), skip_group_check=True,
                    )
                for hh in range(2):
                    prh = pr[:, hh * 512 + coff: hh * 512 + 512]
                    nc.tensor.matmul(
                        cs_t[hh * 64:(hh + 1) * 64, 512 + coff: 1024],
                        ones64_t[:], prh, start=first, stop=last,
                        tile_position=(0, hh * 64), skip_group_check=True,
                    )

            def norm_pair(c, p, cs_t):
                rc_t = recp.tile([128, 512], f32, tag="rc", name=f"rc{c}{p}")
                nc.vector.reciprocal_approx_fast(rc_t[:], cs_t[:, 512:1024])
                nc.vector.tensor_mul(ctxT[:, p * S + c * 512: p * S + (c + 1) * 512],
                                     cs_t[:, 0:512], rc_t[:])

            # --- static weave plan ----------------------------------------
            # bg[i] = list of zero-arg emitters run in tile slot i (global
            # index over the flat stream), placed per the dependency margins
            # worked out in the header comment
            def u_qk(name, src, w_t, dst, bias_t, n, m, half):
                return lambda: qk_proj_half(name, src, w_t, dst, bias_t, n, m, half)

            def u_v(j):
                return lambda: v_proj_j(j)

            def u_o2(c, dc0):
                return lambda: (oproj_dc(c, dc0), oproj_dc(c, dc0 + 1))

            tiles = []
            chunk_base = {}
            for c in range(NQC):
                nj = 4 * c + 4 if causal else NJ
                chunk_base[c] = len(tiles)
                for p in range(2):
                    for j in range(nj):
                        tiles.append((c, p, j, nj))

            bg = [[] for _ in range(len(tiles))]

            def place(c, s, fn):
                bg[chunk_base[c] + s].append(fn)

            def qk_units(name, src, w_t, dst, bias_t, n, m):
                return [u_qk(name, src, w_t, dst, bias_t, n, m, 0),
                        u_qk(name, src, w_t, dst, bias_t, n, m, 1)]

            if causal:
                # chunk 0: qproj(0,1) s0-1; kproj(0,1)+vproj(0,1) s2-3;
                # vproj(2,3) s4-5; qproj(1,0) s6-7
                for s, fn in enumerate(qk_units("q", qb, wq_t, qpT, bq_t, 0, 1)):
                    place(0, s, fn)
                for s, fn in enumerate(qk_units("k", kb, wk_t, kpT, bk_t, 0, 1)):
                    place(0, 2 + s, fn)
                place(0, 2, u_v(0)); place(0, 3, u_v(1))
                place(0, 4, u_v(2)); place(0, 5, u_v(3))
                for s, fn in enumerate(qk_units("q", qb, wq_t, qpT, bq_t, 1, 0)):
                    place(0, 6 + s, fn)
                for c in range(1, NQC):
                    # qproj(c,1) s0-1; kproj(c,0) s2-3.  qproj(3,1) moves to
                    # chunk 2's free ACT-paced slots: c3's partA is PE-bound
                    for s, fn in enumerate(qk_units("q", qb, wq_t, qpT, bq_t, c, 1)):
                        if c == NQC - 1:
                            place(c - 1, 16 + s, fn)
                        else:
                            place(c, s, fn)
                    for s, fn in enumerate(qk_units("k", kb, wk_t, kpT, bk_t, c, 0)):
                        place(c, 2 + s, fn)
                    # vproj(4c..4c+3) at the diag slots s=4c..4c+3
                    for d in range(4):
                        place(c, 4 * c + d, u_v(4 * c + d))
                    # kproj(c,1) at pair1 slots 0-1
                    for s, fn in enumerate(qk_units("k", kb, wk_t, kpT, bk_t, c, 1)):
                        place(c, (4 * c + 4) + s, fn)
                    # oproj(c-1): weave into the attention-only (ACT-paced)
                    # stretches where the PE has per-tile slack, not into the
                    # already PE-bound partA slots
                    ob = 12 if c == 1 else ((4 * c + 4) + 8)
                    for u in range(4):
                        place(c, ob + u, u_o2(c - 1, 2 * u))
                    # qproj(c+1,0) late in the chunk
                    if c + 1 < NQC:
                        qb_s = 10 if c == 1 else 14
                        for s, fn in enumerate(qk_units("q", qb, wq_t, qpT, bq_t, c + 1, 0)):
                            place(c, qb_s + s, fn)
            else:
                # non-causal cold path: all projections upfront, weave only
                # the output projections
                for n in range(1, NQC):
                    for m in range(2):
                        for fn in qk_units("q", qb, wq_t, qpT, bq_t, n, m):
                            fn()
                    for m in range(2):
                        for fn in qk_units("k", kb, wk_t, kpT, bk_t, n, m):
                            fn()
                    for j in range(4 * n, 4 * n + 4):
                        v_proj_j(j)
                for c in range(1, NQC):
                    for u in range(4):
                        place(c, 2 + u, u_o2(c - 1, 2 * u))

            # --- prefix: first projections (chunk 0 deps) ------------------
            for m in ((0,) if causal else (0, 1)):
                for fn in qk_units("q", qb, wq_t, qpT, bq_t, 0, m):
                    fn()
                for fn in qk_units("k", kb, wk_t, kpT, bk_t, 0, m):
                    fn()
            if not causal:
                for j in range(4):
                    v_proj_j(j)

            # --- flat software-pipelined tile stream -----------------------
            LAG = 2
            cs_tiles = {}
            pend = deque()

            def do_pv(ent):
                (c, p, j, nj), pr = ent
                cs_t = cs_tiles[(c, p)]
                attn_j_pv(c, p, j, nj, cs_t, pr)
                if j == nj - 1:
                    norm_pair(c, p, cs_t)

            for idx, (c, p, j, nj) in enumerate(tiles):
                if j == 0:
                    cs_tiles[(c, p)] = csp.tile([128, 1024], f32, tag="cs", name=f"cs{c}{p}")
                pr = attn_j_sc(c, p, j)
                for fn in bg[idx]:
                    fn()
                pend.append(((c, p, j, nj), pr))
                if len(pend) > LAG:
                    do_pv(pend.popleft())
            while pend:
                do_pv(pend.popleft())

            # --- tail: last chunk's output projection ----------------------
            # softmax is finished, so the score-psum banks are free: run the
            # final 8 dc groups through them (2 dc per [128,1024] tile, 4 in
            # flight) so the matmuls stream back-to-back instead of waiting
            # on the 2-deep projection pool's staging copies
            tps = None
            for dc in range(KC):
                if dc % 2 == 0:
                    tps = scp.tile([128, 1024], f32, tag="sc", name=f"otail{dc}")
                oproj_dc(NQC - 1, dc, ops=tps[:, (dc % 2) * 512:(dc % 2 + 1) * 512])

    nc.compile()
    return nc


def _get_nc(causal: bool):
    if causal not in _CACHE:
        _CACHE[causal] = _build(causal)
    return _CACHE[causal]


def _pack_w(w):
    # [D, DG] -> SBUF layout [128, KC*DG]: chunk kc of 128 rows side by side
    return np.ascontiguousarray(w.reshape(KC, 128, DG).transpose(1, 0, 2).reshape(128, KC * DG)).astype(BF16)


def _pack_w_qk(w):
    # [D, DG] -> m-major [128, 2, KC*128]: pair m's weights contiguous
    return np.ascontiguousarray(
        w.reshape(KC, 128, 2, 128).transpose(1, 2, 0, 3)).astype(BF16).reshape(128, 2, KC * 128)


def _pack_seq(xT):
    # [D, S] -> [128, qtr, kc*512]: per-partition-contiguous quarter slabs
    return np.ascontiguousarray(
        xT.reshape(KC, 128, NQC, 512).transpose(1, 2, 0, 3)).astype(BF16).reshape(128, NQC, KC * 512)


def make_in_maps(q, k, v, w_q, b_q, w_k, b_k, w_v, b_v, w_o):
    tri_keep = (np.arange(128)[:, None] <= np.arange(128)[None, :]).astype(BF16)
    qP = [_pack_seq(q[b].T) for b in range(B)]
    kP = [_pack_seq(k[b].T) for b in range(B)]
    vP = [_pack_seq(v[b].T) for b in range(B)]
    in_maps = []
    for core in range(NCORES):
        b, g = core // G, core % G
        sl = slice(g * DG, (g + 1) * DG)
        woTg = np.ascontiguousarray(w_o[:, sl].T)  # [DG, D]
        in_maps.append({
            "qT": qP[b], "kT": kP[b], "vT": vP[b],
            "wqT": _pack_w_qk(np.ascontiguousarray(w_q[sl, :].T)),
            "wkT": _pack_w_qk(np.ascontiguousarray(w_k[sl, :].T)),
            "wvT": _pack_w(np.ascontiguousarray(w_v[sl, :].T)),
            "woT": np.ascontiguousarray(
                woTg.reshape(2, 128, D).transpose(1, 0, 2).reshape(128, 2 * D)).astype(BF16),
            "bq": np.ascontiguousarray(b_q[sl].reshape(2, 128).T).astype(np.float32),
            "bk": np.ascontiguousarray(b_k[sl].reshape(2, 128).T).astype(np.float32),
            "bv": np.ascontiguousarray(b_v[None, sl]).astype(BF16),
            "tri": tri_keep,
        })
    return in_maps


def _reference_numpy(q, k, v, mask, w_q, b_q, w_k, b_k, w_v, b_v, w_o, b_o):
    qp = q @ w_q.T + b_q
    kp = k @ w_k.T + b_k
    vv = v @ w_v.T + b_v
    qp = qp.reshape(B, S, H, HD).transpose(0, 2, 1, 3)
    kp = kp.reshape(B, S, H, HD).transpose(0, 2, 1, 3)
    vv = vv.reshape(B, S, H, HD).transpose(0, 2, 1, 3)
    score = np.einsum("bhqd,bhkd->bhqk", qp, kp) / SCALE
    score = np.where(mask, -1e9, score)
    score -= score.max(axis=-1, keepdims=True)
    e = np.exp(score)
    attn = e / e.sum(axis=-1, keepdims=True)
    ctx = np.einsum("bhqk,bhkd->bhqd", attn, vv)
    ctx = ctx.transpose(0, 2, 1, 3).reshape(B, S, D)
    return (ctx @ w_o.T + b_o).astype(np.float32)


def kernel(q, k, v, mask, w_q, b_q, w_k, b_k, w_v, b_v, w_o, b_o):
    from concourse.bass_utils import run_bass_kernel_spmd

    q, k, v = (np.asarray(x, np.float32) for x in (q, k, v))
    mask = np.asarray(mask)
    causal_ref = np.triu(np.ones((S, S), bool), k=1)
    causal = all(np.array_equal(mask[b, 0], causal_ref) for b in range(B))
    if not causal and mask.any():
        # Unexpected mask pattern: fall back to exact numpy (never hit in
        # practice -- setup_inputs always builds the causal mask).
        return _reference_numpy(q, k, v, mask, w_q, b_q, w_k, b_k, w_v, b_v, w_o, b_o)

    nc = _get_nc(causal)
    in_maps = make_in_maps(q, k, v, w_q, b_q, w_k, b_k, w_v, b_v, w_o)
    res = run_bass_kernel_spmd(nc, in_maps, core_ids=list(range(NCORES)))

    out = np.zeros((B, S, D), np.float32)
    for core in range(NCORES):
        b = core // G
        out[b] += res.results[core]["out"].T.astype(np.float32)
    out += np.asarray(b_o, np.float32)
    return out
